# revision 1
# baseline (speedup 1.0000x reference)
"""Trainium2 Bass kernel for a 2-layer GCN + question-broadcast log_softmax.

Reference computation (see problem statement):
    h1  = relu(gcnconv(x, W1, b1));  h2 = gcnconv(h1, W2, b2)
    out[b, i, :] = log_softmax(h2[i, :] + q[b, :], axis=nodes)
q[b, :] is constant along the softmax axis, so it cancels exactly:
    out[b] = h2 - logsumexp(h2, axis=0)   (identical for every b)
h2 is O(0.1) for this graph (symmetric normalization), so logsumexp is
computed without the max shift: one AllReduce(add) of the per-feature
exp-sums.

Strategy (8 NeuronCores, 1D graph partition by destination node):
  * Nodes are sharded 12500/core (padded to 12544 = 98 windows x 128).
  * The per-core node range is split into 4 chunks (3200/3200/3200/2944
    rows).  The AllGather of y = (x@W)*dinv runs per chunk, so later
    chunks transfer while message passing / the next pipeline stage
    already runs.  Global row ids are chunk-major so each AllGather
    output slice is contiguous; gather indices stay within one chunk's
    <=25600-row range (int16).
  * Per core, incident edges (by dest) are bucketed into (window, chunk)
    cells; window = 128 consecutive local dest nodes.  Blocks of BLKW=4
    windows share ONE dma_gather + ONE indicator build per chunk, cutting
    instruction counts ~4x.  Within a cell, edges are sorted by source
    row so the gather walks HBM in ascending order.
  * Segment-sum into each window is a tensor-engine matmul with an
    indicator matrix S[msg, dest] = (iota == dest) built on the vector
    engine; PSUM accumulates a [128 dest x 128 feat] tile per window
    (the 4 windows of a block accumulate in parallel PSUM banks while
    quarters stream in).
  * x is staged transposed in bf16; weights are bf16 (4x PE throughput).
  * h2 stays resident in SBUF; the epilogue subtracts ln(sum exp) after a
    single AllReduce and writes the output with one batched DMA.

kernel() is self-contained: it hardcodes shapes, preprocesses the integer
graph structure on host (partitioning/sorting/padding only), compiles the
Bass program once per edge-structure, and runs it on 8 cores via
run_bass_kernel_spmd.
"""

import os
import sys
import hashlib
import numpy as np

sys.path.insert(0, "/opt/trn_rl_repo")

import ml_dtypes  # noqa: E402

BF16 = ml_dtypes.bfloat16

TILE = 128
BLKW = 4          # dest windows per gather block (PSUM-bank limited)
NCHUNK = 4        # AllGather chunks (= gather index ranges)


class Cfg:
    def __init__(self, n_nodes, n_edges, d=128, ncores=8):
        assert n_nodes % ncores == 0
        self.N = n_nodes
        self.E = n_edges
        self.D = d
        self.NC = ncores
        self.NPC = n_nodes // ncores                    # real nodes per core
        self.W = (self.NPC + TILE - 1) // TILE          # windows per core
        self.NPCP = self.W * TILE                       # padded nodes per core
        self.last_w = self.NPC - (self.W - 1) * TILE    # dests in last window
        # chunk-major decomposition of the padded local node range;
        # chunk boundaries on window multiples, global chunk <= 32768 rows
        base = (self.W // NCHUNK + 1) * TILE            # 3200 for W=98
        offs, rows = [], []
        o = 0
        for k in range(NCHUNK):
            r = min(base, self.NPCP - o)
            offs.append(o)
            rows.append(r)
            o += r
        assert o == self.NPCP
        assert all(ncores * r <= 32768 for r in rows)
        self.ch_off = offs                              # local row offsets
        self.ch_rows = rows
        self.ch_last_w = [min((off + r) // TILE, self.W) - 1
                          for off, r in zip(offs, rows)]
        self.NB = (self.W + BLKW - 1) // BLKW           # gather blocks


FULL = Cfg(100000, 3200000)

DEST_SENTINEL = 200.0  # compare-miss value for padded message slots


def _stream_layout(cfg, T):
    """Tile offsets for the (block, chunk, window) stream order."""
    W = cfg.W
    tile_off = {}
    bq_off = {}
    off = 0
    for b in range(cfg.NB):
        ws = range(b * BLKW, min((b + 1) * BLKW, W))
        for q in range(NCHUNK):
            s = off
            for w in ws:
                tile_off[(w, q)] = off
                off += T[w][q]
            bq_off[(b, q)] = (s, off - s)
    return tile_off, bq_off, off


# --------------------------------------------------------------------------
# host-side integer preprocessing: partition, bucket, sort, pad
# --------------------------------------------------------------------------

def prepare(cfg, x, edge_index, W1, b1, W2, b2, fake_idx=None):
    N, D, NC = cfg.N, cfg.D, cfg.NC
    NPC, NPCP, W = cfg.NPC, cfg.NPCP, cfg.W
    NQ = NCHUNK
    ch_off = np.asarray(cfg.ch_off)
    ch_rows = np.asarray(cfg.ch_rows)

    row = np.asarray(edge_index[0], dtype=np.int64)
    col = np.asarray(edge_index[1], dtype=np.int64)

    deg = np.bincount(col, minlength=N).astype(np.float32) + 1.0

    core_d = col // NPC
    wl = (col % NPC) // TILE
    drel = (col % NPC) % TILE
    core_r = row // NPC
    lr = row % NPC
    q = np.minimum(lr // int(ch_rows[0]), NQ - 1)
    lidx = core_r * ch_rows[q] + (lr - ch_off[q])        # row within chunk
    blk = wl // BLKW

    # stream order: core, block, chunk, window, source-row
    order = np.lexsort((lidx, wl, q, blk, core_d))
    core_s, wl_s, q_s, drel_s, lidx_s = (
        core_d[order], wl[order], q[order], drel[order], lidx[order])

    # per (core, window, chunk) cell counts -> shared tile table T
    cell = (core_s * W + wl_s) * NQ + q_s
    counts = np.bincount(cell, minlength=NC * W * NQ).reshape(NC, W, NQ)
    T = np.ceil(counts.max(axis=0) / TILE).astype(np.int64)  # [W, NQ]
    Ttup = tuple(map(tuple, T.tolist()))
    tile_off, bq_off, NTT = _stream_layout(cfg, T)
    L = NTT * TILE

    # slot position for every edge: cell start + rank within (core, cell)
    b0 = np.flatnonzero(np.r_[True, cell[1:] != cell[:-1]])
    grp_start = np.repeat(b0, np.diff(np.r_[b0, len(cell)]))
    rank = np.arange(len(cell)) - grp_start
    cell_slot0 = np.zeros((W, NQ), dtype=np.int64)
    for (w, qq), t0 in tile_off.items():
        cell_slot0[w, qq] = t0 * TILE
    pos = cell_slot0[wl_s, q_s] + rank

    idx_arr = np.zeros((NC, L), dtype=np.int16)          # pad -> row 0 (valid)
    dest_arr = np.full((NC, L), DEST_SENTINEL, dtype=np.float32)
    if fake_idx == "seq":  # timing probe: dense ascending gather indices
        idx_arr[core_s, pos] = rank.astype(np.int16)
    else:
        idx_arr[core_s, pos] = lidx_s.astype(np.int16)
    dest_arr[core_s, pos] = drel_s.astype(np.float32)

    # wrapped int16 index layout for dma_gather: G[p, s] = idx[s*16 + p%16],
    # replicated across the 8 groups of 16 partitions
    idx_w = idx_arr.reshape(NC, L // 16, 16).transpose(0, 2, 1)   # [NC,16,L/16]
    idx_w = np.tile(idx_w, (1, 8, 1)).copy()                      # [NC,128,L/16]

    # dest stream transposed: dest_t[p, t] = dest[t*128 + p]
    dest_t = dest_arr.reshape(NC, NTT, TILE).transpose(0, 2, 1).astype(BF16).copy()

    # per-core padded node data; x transposed to [D, NPCP] bf16
    x = np.asarray(x, dtype=np.float32)
    x_t = np.zeros((NC, D, NPCP), dtype=BF16)
    deg_sh = np.ones((NC, NPCP), dtype=np.float32)
    for c in range(NC):
        x_t[c, :, :NPC] = x[c * NPC:(c + 1) * NPC].T.astype(BF16)
        deg_sh[c, :NPC] = deg[c * NPC:(c + 1) * NPC]
    deg_t = deg_sh.reshape(NC, W, TILE).transpose(0, 2, 1).copy()  # [NC,128,W]

    iota = np.broadcast_to(np.arange(TILE, dtype=np.float32), (TILE, TILE))
    iota = iota.astype(BF16).copy()

    W1b = np.asarray(W1, np.float32).astype(BF16)
    W2b = np.asarray(W2, np.float32).astype(BF16)
    b1r = np.asarray(b1, np.float32).reshape(1, D)
    b2r = np.asarray(b2, np.float32).reshape(1, D)

    in_maps = []
    for c in range(NC):
        in_maps.append({
            "x_t": x_t[c],
            "deg_t": deg_t[c],
            "w1": W1b, "w2": W2b, "b1r": b1r, "b2r": b2r,
            "iota": iota,
            "idx_w": idx_w[c],
            "dest_t": dest_t[c],
        })

    meta = {"T": Ttup, "NTT": NTT}
    return in_maps, meta


# --------------------------------------------------------------------------
# Bass program
# --------------------------------------------------------------------------

def build(cfg, meta, stage=5, repeat=1, sim_local=False, layer_mode="full",
          gather_f32=False, nqueues=4, sp1024=True, scratch=16384,
          mbufs=2, s_split=False, qrr=True, spt=4):
    import concourse.bacc as bacc
    import concourse.mybir as mybir
    from concourse.tile import TileContext
    from concourse.masks import make_identity

    f32 = mybir.dt.float32
    bf16 = mybir.dt.bfloat16
    i16 = mybir.dt.int16
    AF = mybir.ActivationFunctionType
    OP = mybir.AluOpType

    D, W, NPCP, NC = cfg.D, cfg.W, cfg.NPCP, cfg.NC
    NQ = NCHUNK
    NB = cfg.NB
    T = meta["T"]
    NTT = meta["NTT"]
    L16 = NTT * TILE // 16
    tile_off, bq_off, _ntt = _stream_layout(cfg, T)
    assert _ntt == NTT
    maxqt = max(n for (_, n) in bq_off.values()) if NTT else 1
    maxbt = max(sum(bq_off[(b, q)][1] for q in range(NQ)) for b in range(NB))
    last_q = {w: max((q for q in range(NQ) if T[w][q]), default=-1)
              for w in range(W)}

    assert not gather_f32 or layer_mode == "gather_only"
    ydt_np = "float32" if gather_f32 else "bfloat16"
    nc = bacc.Bacc(None, target_bir_lowering=False, debug=False,
                   num_swdge_queues=nqueues,
                   dynamic_dma_scratch_size=scratch)

    x_t_in = nc.dram_tensor("x_t", [D, NPCP], bf16, kind="ExternalInput")
    deg_t = nc.dram_tensor("deg_t", [TILE, W], f32, kind="ExternalInput")
    w1 = nc.dram_tensor("w1", [D, D], bf16, kind="ExternalInput")
    w2 = nc.dram_tensor("w2", [D, D], bf16, kind="ExternalInput")
    b1r = nc.dram_tensor("b1r", [1, D], f32, kind="ExternalInput")
    b2r = nc.dram_tensor("b2r", [1, D], f32, kind="ExternalInput")
    iota_in = nc.dram_tensor("iota", [TILE, TILE], bf16, kind="ExternalInput")
    idx_w = nc.dram_tensor("idx_w", [TILE, L16], i16, kind="ExternalInput")
    dest_t = nc.dram_tensor("dest_t", [TILE, NTT], bf16, kind="ExternalInput")

    out_sh = nc.dram_tensor("out_sh", [NPCP, D], f32, kind="ExternalOutput")

    ydt = f32 if gather_f32 else bf16
    y1_sh = nc.dram_tensor("y1_sh", [NPCP, D], ydt)
    y2_sh = nc.dram_tensor("y2_sh", [NPCP, D], ydt)
    y1_full = nc.dram_tensor("y1_full", [NC * NPCP, D], ydt, addr_space="Shared")
    y2_full = nc.dram_tensor("y2_full", [NC * NPCP, D], ydt, addr_space="Shared")
    red_in2 = nc.dram_tensor("red_in2", [1, D], f32)
    red_sum = nc.dram_tensor("red_sum", [1, D], f32, addr_space="Shared")

    groups = [list(range(NC))]

    def ag_chunk(y_sh, y_full_t, k):
        o, r = cfg.ch_off[k], cfg.ch_rows[k]
        if sim_local:
            nc.sync.dma_start(out=y_full_t[NC * o:NC * o + r, :],
                              in_=y_sh[o:o + r, :])
        else:
            nc.gpsimd.collective_compute(
                "AllGather", mybir.AluOpType.bypass, replica_groups=groups,
                ins=[y_sh[o:o + r, :].opt()],
                outs=[y_full_t[NC * o:NC * (o + r), :].opt()])

    with TileContext(nc) as tc:
        with (
            tc.tile_pool(name="const", bufs=1) as constp,
            tc.tile_pool(name="slp", bufs=1) as slp,
            tc.tile_pool(name="h2p", bufs=1) as h2p,
            tc.tile_pool(name="io", bufs=3) as iop,
            tc.tile_pool(name="msg", bufs=mbufs) as msgp,
            tc.tile_pool(name="sp", bufs=mbufs) as sp,
            tc.tile_pool(name="flush", bufs=3) as flp,
            tc.tile_pool(name="acc", bufs=1) as accp,
            tc.tile_pool(name="mm", bufs=BLKW, space="PSUM") as mmp,
            tc.tile_pool(name="hwp", bufs=2, space="PSUM") as hwp,
            tc.tile_pool(name="tr", bufs=2, space="PSUM") as trp,
        ):
            # ---------------- constants ----------------
            ident = constp.tile([TILE, TILE], f32)
            make_identity(nc, ident[:])
            iota_t = constp.tile([TILE, TILE], bf16)
            nc.sync.dma_start(out=iota_t[:], in_=iota_in[:, :])
            w1_t = constp.tile([D, D], bf16)
            nc.sync.dma_start(out=w1_t[:], in_=w1[:, :])
            w2_t = constp.tile([D, D], bf16)
            nc.sync.dma_start(out=w2_t[:], in_=w2[:, :])
            ones_row = constp.tile([1, TILE], f32)
            nc.vector.memset(ones_row[:], 1.0)

            # bias rows broadcast to [128, D] via K=1 matmul
            b_bc = []
            for name, bt in (("b1", b1r), ("b2", b2r)):
                brow = constp.tile([1, D], f32, tag=f"{name}row")
                nc.sync.dma_start(out=brow[:], in_=bt[:, :])
                bps = trp.tile([TILE, D], f32, tag="tr")
                nc.tensor.matmul(bps[:], lhsT=ones_row[:], rhs=brow[:],
                                 start=True, stop=True)
                bsb = constp.tile([TILE, D], f32, tag=f"{name}bc")
                nc.vector.tensor_copy(out=bsb[:], in_=bps[:])
                b_bc.append(bsb)
            b1_bc, b2_bc = b_bc

            # dinv / dinv^2 per local node, window-major [128, W]
            degs = constp.tile([TILE, W], f32)
            nc.sync.dma_start(out=degs[:], in_=deg_t[:, :])
            rdeg = constp.tile([TILE, W], f32)
            nc.vector.reciprocal(rdeg[:], degs[:])
            dinv = constp.tile([TILE, W], f32)
            nc.scalar.sqrt(dinv[:], rdeg[:])
            dinv2 = constp.tile([TILE, W], f32)
            nc.vector.tensor_tensor(out=dinv2[:], in0=dinv[:], in1=dinv[:],
                                    op=OP.mult)

            # self-loop + bias term, kept resident in SBUF: [128, W*128]
            sl = slp.tile([TILE, W * D], f32)
            # h2 resident in SBUF: [128, W*128]
            h2b = h2p.tile([TILE, W * D], f32)

            do_gather = layer_mode != "compute_only"
            do_compute = layer_mode != "gather_only"
            qcnt = [0]
            if not do_gather:
                msg0 = constp.tile([TILE, TILE], bf16, tag="msg0")
                nc.vector.memset(msg0[:], 0.0)

            for _rep in range(repeat):
             # per-window exp-sums; reduced once in the epilogue
             seb = accp.tile([TILE, W], f32, tag="sm")

             # ---------------- prologue: xw, y1, sl1 ----------------
             agq = [0]  # next AG chunk to fire

             def fire_chunks(y_sh, y_full_t, wend, enable):
                 while agq[0] < NQ and cfg.ch_last_w[agq[0]] <= wend:
                     if enable:
                         ag_chunk(y_sh, y_full_t, agq[0])
                     agq[0] += 1

             for b in range(NB):
                 ws = list(range(b * BLKW, min((b + 1) * BLKW, W)))
                 nw = len(ws)
                 xb = iop.tile([TILE, BLKW * D], bf16, tag="xb")
                 nc.sync.dma_start(
                     out=xb[:, :nw * D],
                     in_=x_t_in[:, ws[0] * TILE:(ws[-1] + 1) * TILE])
                 y1b = flp.tile([TILE, BLKW * D], ydt, tag="yb")
                 for i, w in enumerate(ws):
                     xw_ps = hwp.tile([TILE, D], f32, tag="hw")
                     nc.tensor.matmul(xw_ps[:], lhsT=xb[:, i * D:(i + 1) * D],
                                      rhs=w1_t[:], start=True, stop=True)
                     # sl1 = xw * dinv2 + b1
                     nc.vector.scalar_tensor_tensor(
                         out=sl[:, w * D:(w + 1) * D], in0=xw_ps[:],
                         scalar=dinv2[:, w:w + 1], in1=b1_bc[:],
                         op0=OP.mult, op1=OP.add)
                     # y1 = xw * dinv (bf16)
                     nc.scalar.activation(y1b[:, i * D:(i + 1) * D], xw_ps[:],
                                          AF.Copy, scale=dinv[:, w:w + 1])
                 nc.sync.dma_start(
                     out=y1_sh[ws[0] * TILE:(ws[-1] + 1) * TILE, :]
                         .rearrange("(w p) d -> p w d", p=TILE),
                     in_=y1b[:, :nw * D].rearrange("p (w d) -> p w d", d=D))
                 fire_chunks(y1_sh, y1_full, ws[-1], stage >= 1)

             # ---------------- message-passing layers ----------------
             def layer(y_full_t, first):
                 agq[0] = 0
                 for b in range(NB):
                     ws = list(range(b * BLKW, min((b + 1) * BLKW, W)))
                     nw = len(ws)
                     bstart = bq_off[(b, 0)][0]
                     bnt = sum(bq_off[(b, q)][1] for q in range(NQ))
                     it = iop.tile([TILE, maxbt * 8], i16, tag="idx")
                     nc.sync.dma_start(
                         out=it[:, :bnt * 8],
                         in_=idx_w[:, bstart * 8:(bstart + bnt) * 8])
                     dt = iop.tile([TILE, maxbt], bf16, tag="dst")
                     nc.sync.dma_start(
                         out=dt[:, :bnt],
                         in_=dest_t[:, bstart:bstart + bnt])
                     ps = {w: mmp.tile([TILE, D], f32, tag="agg",
                                       name=f"ps{w % BLKW}") for w in ws}
                     started = {w: False for w in ws}
                     for q in range(NQ):
                         qs, qnt = bq_off[(b, q)]
                         if qnt == 0:
                             continue
                         loc0 = qs - bstart
                         if do_compute:
                             st = sp.tile([TILE, maxqt * TILE], bf16, tag="S")
                         if do_gather:
                             mt = msgp.tile([TILE, maxqt * TILE], ydt,
                                            tag="msg")
                             step = spt if sp1024 else qnt
                             for c0 in range(0, qnt, step):
                                 cn = min(step, qnt - c0)
                                 nidx = cn * TILE
                                 nc.gpsimd.dma_gather(
                                     out_ap=mt[:, c0 * TILE:(c0 + cn) * TILE]
                                         .rearrange("p (t e) -> p t e",
                                                    e=TILE),
                                     in_ap=y_full_t[
                                         NC * cfg.ch_off[q]:
                                         NC * (cfg.ch_off[q] +
                                               cfg.ch_rows[q]), :],
                                     idxs_ap=it[:, (loc0 + c0) * 8:
                                                (loc0 + c0 + cn) * 8],
                                     num_idxs=nidx,
                                     num_idxs_reg=nidx,
                                     elem_size=D,
                                     single_packet=(nidx <= 1024),
                                     queue_num=(qcnt[0] % nqueues if qrr
                                                else (b * NQ + q + c0)
                                                % nqueues),
                                 )
                                 qcnt[0] += 1
                         if do_compute:
                             # S[p, t, j] = (iota[j] == dest[p, t])
                             seng = (nc.gpsimd if s_split and
                                     (b * NQ + q) % 3 == 2 else nc.vector)
                             seng.tensor_tensor(
                                 out=st[:, :qnt * TILE]
                                     .rearrange("p (t e) -> p t e", e=TILE),
                                 in0=iota_t[:].unsqueeze(1)
                                     .to_broadcast([TILE, qnt, TILE]),
                                 in1=dt[:, loc0:loc0 + qnt].unsqueeze(2)
                                     .to_broadcast([TILE, qnt, TILE]),
                                 op=OP.is_equal)
                             for w in ws:
                                 nt = T[w][q]
                                 if nt == 0:
                                     continue
                                 lo = tile_off[(w, q)] - qs
                                 for t in range(nt):
                                     rhs = (mt[:, (lo + t) * TILE:
                                               (lo + t + 1) * TILE]
                                            if do_gather else msg0[:])
                                     nc.tensor.matmul(
                                         ps[w][:],
                                         lhsT=st[:, (lo + t) * TILE:
                                                 (lo + t + 1) * TILE],
                                         rhs=rhs,
                                         start=not started[w],
                                         stop=(q == last_q[w] and
                                               t == nt - 1))
                                     started[w] = True
                     # ---------------- flush block ----------------
                     if first:
                         y2b = flp.tile([TILE, BLKW * D], ydt, tag="yb")
                     for i, w in enumerate(ws):
                         slw = sl[:, w * D:(w + 1) * D]
                         dw = TILE if w < W - 1 else cfg.last_w
                         if first:
                             h_t = flp.tile([TILE, D], f32, tag="h")
                             if started[w]:
                                 # h = agg * dinv + sl
                                 nc.vector.scalar_tensor_tensor(
                                     out=h_t[:], in0=ps[w][:],
                                     scalar=dinv[:, w:w + 1], in1=slw,
                                     op0=OP.mult, op1=OP.add)
                             else:
                                 nc.vector.tensor_copy(out=h_t[:], in_=slw)
                             # hw2 = relu(h1) @ W2; relu on transposed copy
                             hT_ps = trp.tile([TILE, D], f32, tag="tr")
                             nc.tensor.transpose(hT_ps[:], h_t[:], ident[:])
                             hT = flp.tile([TILE, D], bf16, tag="hT")
                             nc.scalar.activation(hT[:], hT_ps[:], AF.Relu)
                             hw_ps = hwp.tile([TILE, D], f32, tag="hw")
                             nc.tensor.matmul(hw_ps[:], lhsT=hT[:],
                                              rhs=w2_t[:],
                                              start=True, stop=True)
                             # y2 = hw2 * dinv (bf16)
                             nc.scalar.activation(
                                 y2b[:, i * D:(i + 1) * D], hw_ps[:],
                                 AF.Copy, scale=dinv[:, w:w + 1])
                             # sl2 = hw2 * dinv2 + b2 (in place over sl)
                             nc.vector.scalar_tensor_tensor(
                                 out=slw, in0=hw_ps[:],
                                 scalar=dinv2[:, w:w + 1], in1=b2_bc[:],
                                 op0=OP.mult, op1=OP.add)
                         else:
                             # h2 into the resident SBUF buffer
                             h2w = h2b[:, w * D:(w + 1) * D]
                             if started[w]:
                                 nc.vector.scalar_tensor_tensor(
                                     out=h2w, in0=ps[w][:],
                                     scalar=dinv[:, w:w + 1], in1=slw,
                                     op0=OP.mult, op1=OP.add)
                             else:
                                 nc.vector.tensor_copy(out=h2w, in_=slw)
                             # per-feature sum of exp over this window
                             hT_ps = trp.tile([TILE, D], f32, tag="tr")
                             nc.tensor.transpose(hT_ps[:], h2w, ident[:])
                             e_t = flp.tile([TILE, D], f32, tag="e")
                             nc.scalar.activation(e_t[:, :dw],
                                                  hT_ps[:, :dw], AF.Exp,
                                                  accum_out=seb[:, w:w + 1])
                     if first:
                         nc.sync.dma_start(
                             out=y2_sh[ws[0] * TILE:(ws[-1] + 1) * TILE, :]
                                 .rearrange("(w p) d -> p w d", p=TILE),
                             in_=y2b[:, :nw * D]
                                 .rearrange("p (w d) -> p w d", d=D))
                         fire_chunks(y2_sh, y2_full, ws[-1], stage >= 3)

             if stage >= 2:
                 layer(y1_full, True)
             if stage >= 4:
                 layer(y2_full, False)

             if stage < 5:
                 for w in range(W):
                     o_t = flp.tile([TILE, D], f32, tag="o")
                     nc.vector.tensor_copy(out=o_t[:],
                                           in_=sl[:, w * D:(w + 1) * D])
                     nc.sync.dma_start(out=out_sh[w * TILE:(w + 1) * TILE, :],
                                       in_=o_t[:])
             else:
                 # ---------------- log-softmax epilogue ----------------
                 # corr = ln(sum_all exp(h2)) per feature; out = h2 - corr
                 sum_acc = flp.tile([TILE, 1], f32, tag="sacc")
                 nc.vector.tensor_reduce(out=sum_acc[:], in_=seb[:],
                                         axis=mybir.AxisListType.X, op=OP.add)
                 smT_ps = trp.tile([TILE, TILE], f32, tag="tr")
                 nc.tensor.transpose(smT_ps[:1, :TILE], sum_acc[:], ident[:])
                 sm_row = flp.tile([1, D], f32, tag="smrow")
                 nc.vector.tensor_copy(out=sm_row[:], in_=smT_ps[:1, :TILE])
                 nc.sync.dma_start(out=red_in2[:, :], in_=sm_row[:])
                 if sim_local:
                     nc.sync.dma_start(out=red_sum[:, :], in_=red_in2[:, :])
                 else:
                     nc.gpsimd.collective_compute(
                         "AllReduce", mybir.AluOpType.add,
                         replica_groups=groups,
                         ins=[red_in2.ap().opt()], outs=[red_sum.ap().opt()])
                 gsum_row = constp.tile([1, D], f32, tag="gsumrow")
                 nc.sync.dma_start(out=gsum_row[:], in_=red_sum[:, :])
                 corr_row = constp.tile([1, D], f32, tag="corrrow")
                 nc.scalar.activation(corr_row[:], gsum_row[:], AF.Ln)
                 cb_ps = trp.tile([TILE, D], f32, tag="tr")
                 nc.tensor.matmul(cb_ps[:], lhsT=ones_row[:], rhs=corr_row[:],
                                  start=True, stop=True)
                 corr_bc = constp.tile([TILE, D], f32, tag="corrbc")
                 nc.vector.tensor_copy(out=corr_bc[:], in_=cb_ps[:])

                 # subtract in place over h2b, then one batched output DMA
                 nc.vector.tensor_tensor(
                     out=h2b[:].rearrange("p (w d) -> p w d", d=D),
                     in0=h2b[:].rearrange("p (w d) -> p w d", d=D),
                     in1=corr_bc[:].unsqueeze(1).to_broadcast([TILE, W, D]),
                     op=OP.subtract)
                 nc.sync.dma_start(
                     out=out_sh[:, :].rearrange("(w p) d -> p w d", p=TILE),
                     in_=h2b[:].rearrange("p (w d) -> p w d", d=D))

    nc.finalize()
    return nc


# --------------------------------------------------------------------------
# runner
# --------------------------------------------------------------------------

_CACHE = {}


def get_program(cfg, meta, stage=5, repeat=1, layer_mode="full",
                gather_f32=False, nqueues=4, sp1024=True, scratch=16384,
                mbufs=2, s_split=False, qrr=True, spt=4):
    key = (cfg.N, cfg.E, cfg.D, cfg.NC, meta["T"], stage, repeat, layer_mode,
           gather_f32, nqueues, sp1024, scratch, mbufs, s_split, qrr, spt)
    if key not in _CACHE:
        _CACHE[key] = build(cfg, meta, stage=stage, repeat=repeat,
                            layer_mode=layer_mode, gather_f32=gather_f32,
                            nqueues=nqueues, sp1024=sp1024, scratch=scratch,
                            mbufs=mbufs, s_split=s_split, qrr=qrr, spt=spt)
    return _CACHE[key]


def run(cfg, in_maps, meta):
    from concourse.bass_utils import run_bass_kernel_spmd
    nc = get_program(cfg, meta)
    res = run_bass_kernel_spmd(nc, in_maps, list(range(cfg.NC)))
    return [r["out_sh"] for r in res.results]


def _kernel_impl(cfg, x, edge_index, question_embeddings, W1, b1, W2, b2,
                 Wq, bq):
    in_maps, meta = prepare(cfg, x, edge_index, W1, b1, W2, b2)
    outs = run(cfg, in_maps, meta)
    h = np.concatenate([o[:cfg.NPC] for o in outs], axis=0)
    B = np.asarray(question_embeddings).shape[0]
    out = np.broadcast_to(h[None, :, :], (B, cfg.N, cfg.D))
    return np.ascontiguousarray(out, dtype=np.float32)


def kernel(x, edge_index, question_embeddings, W1, b1, W2, b2, Wq, bq):
    cfg = Cfg(x.shape[0], edge_index.shape[1], x.shape[1])
    return _kernel_impl(cfg, np.asarray(x), np.asarray(edge_index),
                        np.asarray(question_embeddings),
                        np.asarray(W1), np.asarray(b1),
                        np.asarray(W2), np.asarray(b2),
                        np.asarray(Wq), np.asarray(bq))



# revision 2
# speedup vs baseline: 1.4559x; 1.4559x over previous
"""Trainium2 Bass kernel for a 2-layer GCN + question-broadcast log_softmax.

Reference computation (see problem statement):
    h1  = relu(gcnconv(x, W1, b1));  h2 = gcnconv(h1, W2, b2)
    out[b, i, :] = log_softmax(h2[i, :] + q[b, :], axis=nodes)
q[b, :] is constant along the softmax axis, so it cancels exactly:
    out[b] = h2 - logsumexp(h2, axis=0)   (identical for every b)
h2 is O(0.1) for this graph (symmetric normalization), so logsumexp is
computed without the max shift: one AllReduce(add) of the per-feature
exp-sums.

Strategy (8 NeuronCores, 1D graph partition by destination node):
  * Nodes are sharded 12500/core (padded to 12544 = 98 windows x 128).
  * The per-core node range is split into 4 chunks (3200/3200/3200/2944
    rows).  The AllGather of y = (x@W)*dinv runs per chunk, so later
    chunks transfer while message passing / the next pipeline stage
    already runs.  Global row ids are chunk-major so each AllGather
    output slice is contiguous; gather indices stay within one chunk's
    <=25600-row range (int16).
  * Per core, incident edges (by dest) are bucketed into (window, chunk)
    cells; window = 128 consecutive local dest nodes.  Blocks of BLKW=4
    windows share ONE dma_gather + ONE indicator build per chunk, cutting
    instruction counts ~4x.  Within a cell, edges are sorted by source
    row so the gather walks HBM in ascending order.
  * Segment-sum into each window is a tensor-engine matmul with an
    indicator matrix S[msg, dest] = (iota == dest) built on the vector
    engine; PSUM accumulates a [128 dest x 128 feat] tile per window
    (the 4 windows of a block accumulate in parallel PSUM banks while
    quarters stream in).
  * x is staged transposed in bf16; weights are bf16 (4x PE throughput).
  * h2 stays resident in SBUF; the epilogue subtracts ln(sum exp) after a
    single AllReduce and writes the output with one batched DMA.

kernel() is self-contained: it hardcodes shapes, preprocesses the integer
graph structure on host (partitioning/sorting/padding only), compiles the
Bass program once per edge-structure, and runs it on 8 cores via
run_bass_kernel_spmd.
"""

import os
import sys
import hashlib
import numpy as np

sys.path.insert(0, "/opt/trn_rl_repo")

import ml_dtypes  # noqa: E402

BF16 = ml_dtypes.bfloat16

TILE = 128
BLKW = 4          # dest windows per gather block (PSUM-bank limited)
NCHUNK = 4        # AllGather chunks (= gather index ranges)


class Cfg:
    def __init__(self, n_nodes, n_edges, d=128, ncores=8):
        assert n_nodes % ncores == 0
        self.N = n_nodes
        self.E = n_edges
        self.D = d
        self.NC = ncores
        self.NPC = n_nodes // ncores                    # real nodes per core
        self.W = (self.NPC + TILE - 1) // TILE          # windows per core
        self.NPCP = self.W * TILE                       # padded nodes per core
        self.last_w = self.NPC - (self.W - 1) * TILE    # dests in last window
        # chunk-major decomposition of the padded local node range;
        # chunk boundaries on window multiples, global chunk <= 32768 rows
        base = (self.W // NCHUNK + 1) * TILE            # 3200 for W=98
        offs, rows = [], []
        o = 0
        for k in range(NCHUNK):
            r = min(base, self.NPCP - o)
            offs.append(o)
            rows.append(r)
            o += r
        assert o == self.NPCP
        assert all(ncores * r <= 32768 for r in rows)
        self.ch_off = offs                              # local row offsets
        self.ch_rows = rows
        self.ch_last_w = [min((off + r) // TILE, self.W) - 1
                          for off, r in zip(offs, rows)]
        self.NB = (self.W + BLKW - 1) // BLKW           # gather blocks


FULL = Cfg(100000, 3200000)

DEST_SENTINEL = 200.0  # compare-miss value for padded message slots


def _stream_layout(cfg, T):
    """Tile offsets for the (block, chunk, window) stream order."""
    W = cfg.W
    tile_off = {}
    bq_off = {}
    off = 0
    for b in range(cfg.NB):
        ws = range(b * BLKW, min((b + 1) * BLKW, W))
        for q in range(NCHUNK):
            s = off
            for w in ws:
                tile_off[(w, q)] = off
                off += T[w][q]
            bq_off[(b, q)] = (s, off - s)
    return tile_off, bq_off, off


# --------------------------------------------------------------------------
# host-side integer preprocessing: partition, bucket, sort, pad
# --------------------------------------------------------------------------

def prepare(cfg, x, edge_index, W1, b1, W2, b2, fake_idx=None):
    N, D, NC = cfg.N, cfg.D, cfg.NC
    NPC, NPCP, W = cfg.NPC, cfg.NPCP, cfg.W
    NQ = NCHUNK
    ch_off = np.asarray(cfg.ch_off)
    ch_rows = np.asarray(cfg.ch_rows)

    row = np.asarray(edge_index[0], dtype=np.int64)
    col = np.asarray(edge_index[1], dtype=np.int64)

    deg = np.bincount(col, minlength=N).astype(np.float32) + 1.0

    core_d = col // NPC
    wl = (col % NPC) // TILE
    drel = (col % NPC) % TILE
    core_r = row // NPC
    lr = row % NPC
    q = np.minimum(lr // int(ch_rows[0]), NQ - 1)
    lidx = core_r * ch_rows[q] + (lr - ch_off[q])        # row within chunk
    blk = wl // BLKW

    # stream order: core, block, chunk, window, source-row
    order = np.lexsort((lidx, wl, q, blk, core_d))
    core_s, wl_s, q_s, drel_s, lidx_s = (
        core_d[order], wl[order], q[order], drel[order], lidx[order])

    # per (core, window, chunk) cell counts -> shared tile table T
    cell = (core_s * W + wl_s) * NQ + q_s
    counts = np.bincount(cell, minlength=NC * W * NQ).reshape(NC, W, NQ)
    T = np.ceil(counts.max(axis=0) / TILE).astype(np.int64)  # [W, NQ]
    Ttup = tuple(map(tuple, T.tolist()))
    tile_off, bq_off, NTT = _stream_layout(cfg, T)
    L = NTT * TILE

    # slot position for every edge: cell start + rank within (core, cell)
    b0 = np.flatnonzero(np.r_[True, cell[1:] != cell[:-1]])
    grp_start = np.repeat(b0, np.diff(np.r_[b0, len(cell)]))
    rank = np.arange(len(cell)) - grp_start
    cell_slot0 = np.zeros((W, NQ), dtype=np.int64)
    for (w, qq), t0 in tile_off.items():
        cell_slot0[w, qq] = t0 * TILE
    pos = cell_slot0[wl_s, q_s] + rank

    idx_arr = np.zeros((NC, L), dtype=np.int16)          # pad -> row 0 (valid)
    dest_arr = np.full((NC, L), DEST_SENTINEL, dtype=np.float32)
    if fake_idx == "seq":  # timing probe: dense ascending gather indices
        idx_arr[core_s, pos] = rank.astype(np.int16)
    else:
        idx_arr[core_s, pos] = lidx_s.astype(np.int16)
    dest_arr[core_s, pos] = drel_s.astype(np.float32)

    # wrapped int16 index layout for dma_gather: G[p, s] = idx[s*16 + p%16],
    # replicated across the 8 groups of 16 partitions
    idx_w = idx_arr.reshape(NC, L // 16, 16).transpose(0, 2, 1)   # [NC,16,L/16]
    idx_w = np.tile(idx_w, (1, 8, 1)).copy()                      # [NC,128,L/16]

    # dest stream transposed: dest_t[p, t] = dest[t*128 + p]
    dest_t = dest_arr.reshape(NC, NTT, TILE).transpose(0, 2, 1).astype(BF16).copy()

    # per-core padded node data; x transposed to [D, NPCP] bf16
    x = np.asarray(x, dtype=np.float32)
    x_t = np.zeros((NC, D, NPCP), dtype=BF16)
    deg_sh = np.ones((NC, NPCP), dtype=np.float32)
    for c in range(NC):
        x_t[c, :, :NPC] = x[c * NPC:(c + 1) * NPC].T.astype(BF16)
        deg_sh[c, :NPC] = deg[c * NPC:(c + 1) * NPC]
    deg_t = deg_sh.reshape(NC, W, TILE).transpose(0, 2, 1).copy()  # [NC,128,W]

    iota = np.broadcast_to(np.arange(TILE, dtype=np.float32), (TILE, TILE))
    iota = iota.astype(BF16).copy()

    W1b = np.asarray(W1, np.float32).astype(BF16)
    W2b = np.asarray(W2, np.float32).astype(BF16)
    b1r = np.asarray(b1, np.float32).reshape(1, D)
    b2r = np.asarray(b2, np.float32).reshape(1, D)

    in_maps = []
    for c in range(NC):
        in_maps.append({
            "x_t": x_t[c],
            "deg_t": deg_t[c],
            "w1": W1b, "w2": W2b, "b1r": b1r, "b2r": b2r,
            "iota": iota,
            "idx_w": idx_w[c],
            "dest_t": dest_t[c],
        })

    meta = {"T": Ttup, "NTT": NTT}
    return in_maps, meta


# --------------------------------------------------------------------------
# Bass program
# --------------------------------------------------------------------------

def build(cfg, meta, stage=5, repeat=1, sim_local=False, layer_mode="full",
          gather_f32=False, nqueues=4, sp1024=True, scratch=16384,
          mbufs=2, s_split=False, qrr=True, spt=8):
    import concourse.bacc as bacc
    import concourse.mybir as mybir
    from concourse.tile import TileContext
    from concourse.masks import make_identity

    f32 = mybir.dt.float32
    bf16 = mybir.dt.bfloat16
    i16 = mybir.dt.int16
    AF = mybir.ActivationFunctionType
    OP = mybir.AluOpType

    D, W, NPCP, NC = cfg.D, cfg.W, cfg.NPCP, cfg.NC
    NQ = NCHUNK
    NB = cfg.NB
    T = meta["T"]
    NTT = meta["NTT"]
    L16 = NTT * TILE // 16
    tile_off, bq_off, _ntt = _stream_layout(cfg, T)
    assert _ntt == NTT
    maxqt = max(n for (_, n) in bq_off.values()) if NTT else 1
    maxbt = max(sum(bq_off[(b, q)][1] for q in range(NQ)) for b in range(NB))
    last_q = {w: max((q for q in range(NQ) if T[w][q]), default=-1)
              for w in range(W)}

    assert not gather_f32 or layer_mode == "gather_only"
    ydt_np = "float32" if gather_f32 else "bfloat16"
    nc = bacc.Bacc(None, target_bir_lowering=False, debug=False,
                   num_swdge_queues=nqueues,
                   dynamic_dma_scratch_size=scratch)

    x_t_in = nc.dram_tensor("x_t", [D, NPCP], bf16, kind="ExternalInput")
    deg_t = nc.dram_tensor("deg_t", [TILE, W], f32, kind="ExternalInput")
    w1 = nc.dram_tensor("w1", [D, D], bf16, kind="ExternalInput")
    w2 = nc.dram_tensor("w2", [D, D], bf16, kind="ExternalInput")
    b1r = nc.dram_tensor("b1r", [1, D], f32, kind="ExternalInput")
    b2r = nc.dram_tensor("b2r", [1, D], f32, kind="ExternalInput")
    iota_in = nc.dram_tensor("iota", [TILE, TILE], bf16, kind="ExternalInput")
    idx_w = nc.dram_tensor("idx_w", [TILE, L16], i16, kind="ExternalInput")
    dest_t = nc.dram_tensor("dest_t", [TILE, NTT], bf16, kind="ExternalInput")

    out_sh = nc.dram_tensor("out_sh", [NPCP, D], f32, kind="ExternalOutput")

    ydt = f32 if gather_f32 else bf16
    y1_sh = nc.dram_tensor("y1_sh", [NPCP, D], ydt)
    y2_sh = nc.dram_tensor("y2_sh", [NPCP, D], ydt)
    y1_full = nc.dram_tensor("y1_full", [NC * NPCP, D], ydt, addr_space="Shared")
    y2_full = nc.dram_tensor("y2_full", [NC * NPCP, D], ydt, addr_space="Shared")
    red_in2 = nc.dram_tensor("red_in2", [1, D], f32)
    red_sum = nc.dram_tensor("red_sum", [1, D], f32, addr_space="Shared")

    groups = [list(range(NC))]

    def ag_chunk(y_sh, y_full_t, k):
        o, r = cfg.ch_off[k], cfg.ch_rows[k]
        if sim_local:
            nc.sync.dma_start(out=y_full_t[NC * o:NC * o + r, :],
                              in_=y_sh[o:o + r, :])
        else:
            nc.gpsimd.collective_compute(
                "AllGather", mybir.AluOpType.bypass, replica_groups=groups,
                ins=[y_sh[o:o + r, :].opt()],
                outs=[y_full_t[NC * o:NC * (o + r), :].opt()])

    with TileContext(nc) as tc:
        with (
            tc.tile_pool(name="const", bufs=1) as constp,
            tc.tile_pool(name="slp", bufs=1) as slp,
            tc.tile_pool(name="h2p", bufs=1) as h2p,
            tc.tile_pool(name="io", bufs=3) as iop,
            tc.tile_pool(name="msg", bufs=mbufs) as msgp,
            tc.tile_pool(name="sp", bufs=mbufs) as sp,
            tc.tile_pool(name="flush", bufs=3) as flp,
            tc.tile_pool(name="acc", bufs=1) as accp,
            tc.tile_pool(name="mm", bufs=BLKW, space="PSUM") as mmp,
            tc.tile_pool(name="hwp", bufs=2, space="PSUM") as hwp,
            tc.tile_pool(name="tr", bufs=2, space="PSUM") as trp,
        ):
            # ---------------- constants ----------------
            ident = constp.tile([TILE, TILE], f32)
            make_identity(nc, ident[:])
            iota_t = constp.tile([TILE, TILE], bf16)
            nc.sync.dma_start(out=iota_t[:], in_=iota_in[:, :])
            w1_t = constp.tile([D, D], bf16)
            nc.sync.dma_start(out=w1_t[:], in_=w1[:, :])
            w2_t = constp.tile([D, D], bf16)
            nc.sync.dma_start(out=w2_t[:], in_=w2[:, :])
            ones_row = constp.tile([1, TILE], f32)
            nc.vector.memset(ones_row[:], 1.0)

            # bias rows broadcast to [128, D] via K=1 matmul
            b_bc = []
            for name, bt in (("b1", b1r), ("b2", b2r)):
                brow = constp.tile([1, D], f32, tag=f"{name}row")
                nc.sync.dma_start(out=brow[:], in_=bt[:, :])
                bps = trp.tile([TILE, D], f32, tag="tr")
                nc.tensor.matmul(bps[:], lhsT=ones_row[:], rhs=brow[:],
                                 start=True, stop=True)
                bsb = constp.tile([TILE, D], f32, tag=f"{name}bc")
                nc.vector.tensor_copy(out=bsb[:], in_=bps[:])
                b_bc.append(bsb)
            b1_bc, b2_bc = b_bc

            # dinv / dinv^2 per local node, window-major [128, W]
            degs = constp.tile([TILE, W], f32)
            nc.sync.dma_start(out=degs[:], in_=deg_t[:, :])
            rdeg = constp.tile([TILE, W], f32)
            nc.vector.reciprocal(rdeg[:], degs[:])
            dinv = constp.tile([TILE, W], f32)
            nc.scalar.sqrt(dinv[:], rdeg[:])
            dinv2 = constp.tile([TILE, W], f32)
            nc.vector.tensor_tensor(out=dinv2[:], in0=dinv[:], in1=dinv[:],
                                    op=OP.mult)

            # self-loop + bias term, kept resident in SBUF: [128, W*128]
            sl = slp.tile([TILE, W * D], f32)
            # h2 resident in SBUF: [128, W*128]
            h2b = h2p.tile([TILE, W * D], f32)

            do_gather = layer_mode != "compute_only"
            do_compute = layer_mode != "gather_only"
            qcnt = [0]
            if not do_gather:
                msg0 = constp.tile([TILE, TILE], bf16, tag="msg0")
                nc.vector.memset(msg0[:], 0.0)

            for _rep in range(repeat):
             # per-window exp-sums; reduced once in the epilogue
             seb = accp.tile([TILE, W], f32, tag="sm")

             # ---------------- prologue: xw, y1, sl1 ----------------
             agq = [0]  # next AG chunk to fire

             def fire_chunks(y_sh, y_full_t, wend, enable):
                 while agq[0] < NQ and cfg.ch_last_w[agq[0]] <= wend:
                     if enable:
                         ag_chunk(y_sh, y_full_t, agq[0])
                     agq[0] += 1

             for b in range(NB):
                 ws = list(range(b * BLKW, min((b + 1) * BLKW, W)))
                 nw = len(ws)
                 xb = iop.tile([TILE, BLKW * D], bf16, tag="xb")
                 nc.sync.dma_start(
                     out=xb[:, :nw * D],
                     in_=x_t_in[:, ws[0] * TILE:(ws[-1] + 1) * TILE])
                 y1b = flp.tile([TILE, BLKW * D], ydt, tag="yb")
                 for i, w in enumerate(ws):
                     xw_ps = hwp.tile([TILE, D], f32, tag="hw")
                     nc.tensor.matmul(xw_ps[:], lhsT=xb[:, i * D:(i + 1) * D],
                                      rhs=w1_t[:], start=True, stop=True)
                     # sl1 = xw * dinv2 + b1
                     nc.vector.scalar_tensor_tensor(
                         out=sl[:, w * D:(w + 1) * D], in0=xw_ps[:],
                         scalar=dinv2[:, w:w + 1], in1=b1_bc[:],
                         op0=OP.mult, op1=OP.add)
                     # y1 = xw * dinv (bf16)
                     nc.scalar.activation(y1b[:, i * D:(i + 1) * D], xw_ps[:],
                                          AF.Copy, scale=dinv[:, w:w + 1])
                 nc.sync.dma_start(
                     out=y1_sh[ws[0] * TILE:(ws[-1] + 1) * TILE, :]
                         .rearrange("(w p) d -> p w d", p=TILE),
                     in_=y1b[:, :nw * D].rearrange("p (w d) -> p w d", d=D))
                 fire_chunks(y1_sh, y1_full, ws[-1], stage >= 1)

             # ---------------- message-passing layers ----------------
             def layer(y_full_t, first):
                 agq[0] = 0
                 for b in range(NB):
                     ws = list(range(b * BLKW, min((b + 1) * BLKW, W)))
                     nw = len(ws)
                     bstart = bq_off[(b, 0)][0]
                     bnt = sum(bq_off[(b, q)][1] for q in range(NQ))
                     it = iop.tile([TILE, maxbt * 8], i16, tag="idx")
                     nc.sync.dma_start(
                         out=it[:, :bnt * 8],
                         in_=idx_w[:, bstart * 8:(bstart + bnt) * 8])
                     dt = iop.tile([TILE, maxbt], bf16, tag="dst")
                     nc.sync.dma_start(
                         out=dt[:, :bnt],
                         in_=dest_t[:, bstart:bstart + bnt])
                     ps = {w: mmp.tile([TILE, D], f32, tag="agg",
                                       name=f"ps{w % BLKW}") for w in ws}
                     started = {w: False for w in ws}
                     for q in range(NQ):
                         qs, qnt = bq_off[(b, q)]
                         if qnt == 0:
                             continue
                         loc0 = qs - bstart
                         if do_compute:
                             st = sp.tile([TILE, maxqt * TILE], bf16, tag="S")
                         if do_gather:
                             mt = msgp.tile([TILE, maxqt * TILE], ydt,
                                            tag="msg")
                             step = spt if sp1024 else qnt
                             for c0 in range(0, qnt, step):
                                 cn = min(step, qnt - c0)
                                 nidx = cn * TILE
                                 nc.gpsimd.dma_gather(
                                     out_ap=mt[:, c0 * TILE:(c0 + cn) * TILE]
                                         .rearrange("p (t e) -> p t e",
                                                    e=TILE),
                                     in_ap=y_full_t[
                                         NC * cfg.ch_off[q]:
                                         NC * (cfg.ch_off[q] +
                                               cfg.ch_rows[q]), :],
                                     idxs_ap=it[:, (loc0 + c0) * 8:
                                                (loc0 + c0 + cn) * 8],
                                     num_idxs=nidx,
                                     num_idxs_reg=nidx,
                                     elem_size=D,
                                     single_packet=(nidx <= 1024),
                                     queue_num=(qcnt[0] % nqueues if qrr
                                                else (b * NQ + q + c0)
                                                % nqueues),
                                 )
                                 qcnt[0] += 1
                         if do_compute:
                             # S[p, t, j] = (iota[j] == dest[p, t])
                             seng = (nc.gpsimd if s_split and
                                     (b * NQ + q) % 3 == 2 else nc.vector)
                             seng.tensor_tensor(
                                 out=st[:, :qnt * TILE]
                                     .rearrange("p (t e) -> p t e", e=TILE),
                                 in0=iota_t[:].unsqueeze(1)
                                     .to_broadcast([TILE, qnt, TILE]),
                                 in1=dt[:, loc0:loc0 + qnt].unsqueeze(2)
                                     .to_broadcast([TILE, qnt, TILE]),
                                 op=OP.is_equal)
                             for w in ws:
                                 nt = T[w][q]
                                 if nt == 0:
                                     continue
                                 lo = tile_off[(w, q)] - qs
                                 for t in range(nt):
                                     rhs = (mt[:, (lo + t) * TILE:
                                               (lo + t + 1) * TILE]
                                            if do_gather else msg0[:])
                                     nc.tensor.matmul(
                                         ps[w][:],
                                         lhsT=st[:, (lo + t) * TILE:
                                                 (lo + t + 1) * TILE],
                                         rhs=rhs,
                                         start=not started[w],
                                         stop=(q == last_q[w] and
                                               t == nt - 1))
                                     started[w] = True
                     # ---------------- flush block ----------------
                     if first:
                         y2b = flp.tile([TILE, BLKW * D], ydt, tag="yb")
                     for i, w in enumerate(ws):
                         slw = sl[:, w * D:(w + 1) * D]
                         dw = TILE if w < W - 1 else cfg.last_w
                         if first:
                             h_t = flp.tile([TILE, D], f32, tag="h")
                             if started[w]:
                                 # h = agg * dinv + sl
                                 nc.vector.scalar_tensor_tensor(
                                     out=h_t[:], in0=ps[w][:],
                                     scalar=dinv[:, w:w + 1], in1=slw,
                                     op0=OP.mult, op1=OP.add)
                             else:
                                 nc.vector.tensor_copy(out=h_t[:], in_=slw)
                             # hw2 = relu(h1) @ W2; relu on transposed copy
                             hT_ps = trp.tile([TILE, D], f32, tag="tr")
                             nc.tensor.transpose(hT_ps[:], h_t[:], ident[:])
                             hT = flp.tile([TILE, D], bf16, tag="hT")
                             nc.scalar.activation(hT[:], hT_ps[:], AF.Relu)
                             hw_ps = hwp.tile([TILE, D], f32, tag="hw")
                             nc.tensor.matmul(hw_ps[:], lhsT=hT[:],
                                              rhs=w2_t[:],
                                              start=True, stop=True)
                             # y2 = hw2 * dinv (bf16)
                             nc.scalar.activation(
                                 y2b[:, i * D:(i + 1) * D], hw_ps[:],
                                 AF.Copy, scale=dinv[:, w:w + 1])
                             # sl2 = hw2 * dinv2 + b2 (in place over sl)
                             nc.vector.scalar_tensor_tensor(
                                 out=slw, in0=hw_ps[:],
                                 scalar=dinv2[:, w:w + 1], in1=b2_bc[:],
                                 op0=OP.mult, op1=OP.add)
                         else:
                             # h2 into the resident SBUF buffer
                             h2w = h2b[:, w * D:(w + 1) * D]
                             if started[w]:
                                 nc.vector.scalar_tensor_tensor(
                                     out=h2w, in0=ps[w][:],
                                     scalar=dinv[:, w:w + 1], in1=slw,
                                     op0=OP.mult, op1=OP.add)
                             else:
                                 nc.vector.tensor_copy(out=h2w, in_=slw)
                             # per-feature sum of exp over this window
                             hT_ps = trp.tile([TILE, D], f32, tag="tr")
                             nc.tensor.transpose(hT_ps[:], h2w, ident[:])
                             e_t = flp.tile([TILE, D], f32, tag="e")
                             nc.scalar.activation(e_t[:, :dw],
                                                  hT_ps[:, :dw], AF.Exp,
                                                  accum_out=seb[:, w:w + 1])
                     if first:
                         nc.sync.dma_start(
                             out=y2_sh[ws[0] * TILE:(ws[-1] + 1) * TILE, :]
                                 .rearrange("(w p) d -> p w d", p=TILE),
                             in_=y2b[:, :nw * D]
                                 .rearrange("p (w d) -> p w d", d=D))
                         fire_chunks(y2_sh, y2_full, ws[-1], stage >= 3)

             if stage >= 2:
                 layer(y1_full, True)
             if stage >= 4:
                 layer(y2_full, False)

             if stage < 5:
                 for w in range(W):
                     o_t = flp.tile([TILE, D], f32, tag="o")
                     nc.vector.tensor_copy(out=o_t[:],
                                           in_=sl[:, w * D:(w + 1) * D])
                     nc.sync.dma_start(out=out_sh[w * TILE:(w + 1) * TILE, :],
                                       in_=o_t[:])
             else:
                 # ---------------- log-softmax epilogue ----------------
                 # corr = ln(sum_all exp(h2)) per feature; out = h2 - corr
                 sum_acc = flp.tile([TILE, 1], f32, tag="sacc")
                 nc.vector.tensor_reduce(out=sum_acc[:], in_=seb[:],
                                         axis=mybir.AxisListType.X, op=OP.add)
                 smT_ps = trp.tile([TILE, TILE], f32, tag="tr")
                 nc.tensor.transpose(smT_ps[:1, :TILE], sum_acc[:], ident[:])
                 sm_row = flp.tile([1, D], f32, tag="smrow")
                 nc.vector.tensor_copy(out=sm_row[:], in_=smT_ps[:1, :TILE])
                 nc.sync.dma_start(out=red_in2[:, :], in_=sm_row[:])
                 if sim_local:
                     nc.sync.dma_start(out=red_sum[:, :], in_=red_in2[:, :])
                 else:
                     nc.gpsimd.collective_compute(
                         "AllReduce", mybir.AluOpType.add,
                         replica_groups=groups,
                         ins=[red_in2.ap().opt()], outs=[red_sum.ap().opt()])
                 gsum_row = constp.tile([1, D], f32, tag="gsumrow")
                 nc.sync.dma_start(out=gsum_row[:], in_=red_sum[:, :])
                 corr_row = constp.tile([1, D], f32, tag="corrrow")
                 nc.scalar.activation(corr_row[:], gsum_row[:], AF.Ln)
                 cb_ps = trp.tile([TILE, D], f32, tag="tr")
                 nc.tensor.matmul(cb_ps[:], lhsT=ones_row[:], rhs=corr_row[:],
                                  start=True, stop=True)
                 corr_bc = constp.tile([TILE, D], f32, tag="corrbc")
                 nc.vector.tensor_copy(out=corr_bc[:], in_=cb_ps[:])

                 # subtract in place over h2b, then one batched output DMA
                 nc.vector.tensor_tensor(
                     out=h2b[:].rearrange("p (w d) -> p w d", d=D),
                     in0=h2b[:].rearrange("p (w d) -> p w d", d=D),
                     in1=corr_bc[:].unsqueeze(1).to_broadcast([TILE, W, D]),
                     op=OP.subtract)
                 nc.sync.dma_start(
                     out=out_sh[:, :].rearrange("(w p) d -> p w d", p=TILE),
                     in_=h2b[:].rearrange("p (w d) -> p w d", d=D))

    nc.finalize()
    return nc


# --------------------------------------------------------------------------
# runner
# --------------------------------------------------------------------------

_CACHE = {}


def get_program(cfg, meta, stage=5, repeat=1, layer_mode="full",
                gather_f32=False, nqueues=4, sp1024=True, scratch=16384,
                mbufs=2, s_split=False, qrr=True, spt=8):
    key = (cfg.N, cfg.E, cfg.D, cfg.NC, meta["T"], stage, repeat, layer_mode,
           gather_f32, nqueues, sp1024, scratch, mbufs, s_split, qrr, spt)
    if key not in _CACHE:
        _CACHE[key] = build(cfg, meta, stage=stage, repeat=repeat,
                            layer_mode=layer_mode, gather_f32=gather_f32,
                            nqueues=nqueues, sp1024=sp1024, scratch=scratch,
                            mbufs=mbufs, s_split=s_split, qrr=qrr, spt=spt)
    return _CACHE[key]


def run(cfg, in_maps, meta):
    from concourse.bass_utils import run_bass_kernel_spmd
    nc = get_program(cfg, meta)
    res = run_bass_kernel_spmd(nc, in_maps, list(range(cfg.NC)))
    return [r["out_sh"] for r in res.results]


def _kernel_impl(cfg, x, edge_index, question_embeddings, W1, b1, W2, b2,
                 Wq, bq):
    in_maps, meta = prepare(cfg, x, edge_index, W1, b1, W2, b2)
    outs = run(cfg, in_maps, meta)
    h = np.concatenate([o[:cfg.NPC] for o in outs], axis=0)
    B = np.asarray(question_embeddings).shape[0]
    out = np.broadcast_to(h[None, :, :], (B, cfg.N, cfg.D))
    return np.ascontiguousarray(out, dtype=np.float32)


def kernel(x, edge_index, question_embeddings, W1, b1, W2, b2, Wq, bq):
    cfg = Cfg(x.shape[0], edge_index.shape[1], x.shape[1])
    return _kernel_impl(cfg, np.asarray(x), np.asarray(edge_index),
                        np.asarray(question_embeddings),
                        np.asarray(W1), np.asarray(b1),
                        np.asarray(W2), np.asarray(b2),
                        np.asarray(Wq), np.asarray(bq))

